# revision 36
# baseline (speedup 1.0000x reference)
"""Trainium2 Bass kernel for a pre-LN transformer block (MHA+RoPE, SiLU FFN).

Sharding: 8 cores; core c handles batch c//4, query block (c%4)*512..+512.
Each core redundantly computes LN1 + K/V for its whole batch (no collectives),
then attention/proj/FFN for its 512 queries. Inputs are column-rolled on the
host so every core's queries are token columns 0:512 of its xT input (SPMD
program identical across cores; RoPE tables rolled to match).

All activations live feature-major ([feature, token]); V is produced row-major
via an acts-stationary matmul so the attention AV contraction needs no
transposes. Softmax runs without max subtraction (scores are O(5) here), with
the denominator accumulated via a ones-column appended to V. RoPE's rotate-half
becomes an adjacent-pair swap (a 32-lane stream_shuffle) by permuting the q/k
weight columns on the host; q.k dot products are permutation-invariant.
LN gains/biases and the V bias are folded into adjacent weights on the host.

v2: x loaded once as bf16 (SWDGE cast-DMA) and LN1 applied in place
((x - m) then * rstd, two broadcast tensor ops, no second DRAM pass);
weights prefetched ahead of use; the attention inner loop computes both
head-halves' scores into one 2-bank PSUM tile (concurrent PE row groups)
and runs a single [128,1024] exp per key tile; PSUM evacuations run on the
DVE; reciprocals use the fast approx DVE op; h1 kept in bf16.
"""
import sys
from contextlib import ExitStack

sys.path.insert(0, "/opt/trn_rl_repo")

import numpy as np
import ml_dtypes

import concourse.bass as bass
import concourse.mybir as mybir
from concourse import bacc
from concourse.tile import TileContext
from concourse.bass_utils import run_bass_kernel_spmd

DIM, HEADS, B, T = 1024, 16, 2, 2048
HD = DIM // HEADS          # 64
NCORES = 8
CPB = NCORES // B          # cores per batch
QBLK = T // CPB            # 512 queries per core
ROPE_THETA = 10000.0
LN_EPS = 1e-5
KT = DIM // 128            # 8 feature tiles over DIM
NCH = T // 512             # 4 column chunks over T
RT = T // 128              # 16 key-row tiles

F32 = mybir.dt.float32
BF16 = mybir.dt.bfloat16
AF = mybir.ActivationFunctionType
OP = mybir.AluOpType

_bf = ml_dtypes.bfloat16


def _ln_finalize(nc, pool, ps_sum, ps_sq, eps_sb, tag):
    """From psum row-sums of x and x^2 over DIM, produce bf16 rstd, -mean
    and -mean*rstd rows ([1, 512]).  Uses 3 f32 slots via in-place reuse."""
    m_row = pool.tile([1, 512], F32, tag=f"{tag}m", name=f"{tag}_m")
    nc.scalar.mul(out=m_row[:], in_=ps_sum[:], mul=1.0 / DIM)
    msq = pool.tile([1, 512], F32, tag=f"{tag}msq", name=f"{tag}_msq")
    nc.scalar.mul(out=msq[:], in_=ps_sq[:], mul=1.0 / DIM)
    var = pool.tile([1, 512], F32, tag=f"{tag}var", name=f"{tag}_var")
    nc.vector.tensor_mul(out=var[:], in0=m_row[:], in1=m_row[:])
    nc.vector.tensor_sub(out=var[:], in0=msq[:], in1=var[:])
    nc.scalar.activation(out=var[:], in_=var[:], func=AF.Sqrt, bias=eps_sb[:])
    nc.vector.reciprocal_approx_fast(out=msq[:], in_=var[:])   # msq <- rstd
    rs_bf = pool.tile([1, 512], BF16, tag=f"{tag}rsbf", name=f"{tag}_rsbf")
    nc.scalar.copy(out=rs_bf[:], in_=msq[:])
    nm_bf = pool.tile([1, 512], BF16, tag=f"{tag}nmbf", name=f"{tag}_nmbf")
    nc.vector.tensor_scalar_mul(out=nm_bf[:], in0=m_row[:], scalar1=-1.0)
    return rs_bf, nm_bf


def _build_program():
    nc = bacc.Bacc("TRN2", target_bir_lowering=False, debug=False, num_devices=NCORES)

    xT = nc.declare_dram_parameter("xT", [DIM, T], F32, isOutput=False)
    cosd = nc.declare_dram_parameter("cosd", [128, T], BF16, isOutput=False)
    sind = nc.declare_dram_parameter("sind", [128, T], BF16, isOutput=False)
    Wq = nc.declare_dram_parameter("Wq", [DIM, DIM], BF16, isOutput=False)
    Wk = nc.declare_dram_parameter("Wk", [DIM, DIM], BF16, isOutput=False)
    Wv = nc.declare_dram_parameter("Wv", [DIM, DIM], BF16, isOutput=False)
    Wp = nc.declare_dram_parameter("Wp", [DIM, DIM], BF16, isOutput=False)
    W1 = nc.declare_dram_parameter("W1", [DIM, 4 * DIM], BF16, isOutput=False)
    W2 = nc.declare_dram_parameter("W2", [4 * DIM, DIM], BF16, isOutput=False)
    bq = nc.declare_dram_parameter("bq", [DIM], F32, isOutput=False)
    bk = nc.declare_dram_parameter("bk", [DIM], F32, isOutput=False)
    bp = nc.declare_dram_parameter("bp", [DIM], F32, isOutput=False)
    b1 = nc.declare_dram_parameter("b1", [4 * DIM], F32, isOutput=False)
    b2 = nc.declare_dram_parameter("b2", [DIM], F32, isOutput=False)
    wsums = nc.declare_dram_parameter("wsums", [1, 3 * DIM], BF16, isOutput=False)
    outT = nc.declare_dram_parameter("outT", [DIM, QBLK], F32, isOutput=True)

    swap_mask = [j ^ 1 for j in range(32)]

    with TileContext(nc) as tc:
        with (
            tc.tile_pool(name="consts", bufs=1) as consts,
            tc.tile_pool(name="h1", bufs=KT) as h1p,
            tc.tile_pool(name="avsb", bufs=KT) as avp,
        ):
            ones_bf = consts.tile([128, 1], BF16)
            nc.vector.memset(ones_bf[:], 1.0)
            ones_row_bf = consts.tile([1, 128], BF16)
            nc.vector.memset(ones_row_bf[:], 1.0)
            eps_sb = consts.tile([1, 1], F32)
            nc.vector.memset(eps_sb[:], LN_EPS)
            bq_sb = consts.tile([128, KT], F32)
            bk_sb = consts.tile([128, KT], F32)
            bp_sb = consts.tile([128, KT], F32)
            b1_sb = consts.tile([128, 4 * KT], F32)
            b2_sb = consts.tile([128, KT], F32)

            h1_tiles = []
            k_tiles, v_tiles, q_tiles = [], [], []
            # ksb/vsb/qsb live from phase 2 through attention, then close so
            # the FFN phases get their SBUF back.
            kvq_ctx = ExitStack()
            ksbp = kvq_ctx.enter_context(tc.tile_pool(name="ksb", bufs=KT))
            vsbp = kvq_ctx.enter_context(tc.tile_pool(name="vsb", bufs=RT))
            qsbp = kvq_ctx.enter_context(tc.tile_pool(name="qsb", bufs=KT))
            with (
                tc.tile_pool(name="xbf", bufs=KT) as xbfp,
                tc.tile_pool(name="cossin", bufs=1) as cossinp,
                tc.tile_pool(name="fold", bufs=1) as foldp,
            ):
                # x loads first (HWDGE fp32 half-rows, cast to bf16 on the
                # idle ScalarE); weight DMAs issue later in program order so
                # x wins the queue.  LN1 is *folded*: Q/K/V matmuls run on
                # raw x; the rstd multiply folds into the rope tables (q/k)
                # and a per-partition V scale; the -mean correction enters
                # as a rank-1 augment matmul using weight column sums.
                xbf_tiles = [xbfp.tile([128, T], BF16, tag="xbf",
                                       name=f"xbf_{k}") for k in range(KT)]
                cos_sb = cossinp.tile([128, T], BF16)
                sin_sb = cossinp.tile([128, T], BF16)
                negm_row = foldp.tile([1, T], BF16)
                rs_col = foldp.tile([128, RT], F32)
                wsums_sb = foldp.tile([1, 3 * DIM], BF16)

                # ---- Phase 1: LN1 stats + fold-table construction ----
                wk_ctx = ExitStack()
                wkp = wk_ctx.enter_context(tc.tile_pool(name="wk", bufs=KT))
                wk_t, wq_t = [], []
                with (
                    tc.tile_pool(name="xsq", bufs=1) as xsqp,
                    tc.tile_pool(name="stats", bufs=1) as statp,
                    tc.tile_pool(name="bcast", bufs=1) as bcastp,
                    tc.tile_pool(name="ps_st", bufs=NCH, space="PSUM") as ps_stp,
                ):
                    ps_sums = [ps_stp.tile([1, 512], F32, tag="ps_sum",
                                           name=f"ps_sum_{n}") for n in range(NCH)]
                    ps_sqs = [ps_stp.tile([1, 512], F32, tag="ps_sq",
                                          name=f"ps_sq_{n}") for n in range(NCH)]
                    for k in range(KT):
                        nc.gpsimd.dma_start(out=xbf_tiles[k][:],
                                            in_=xT[k * 128:(k + 1) * 128, :])
                    nc.sync.dma_start(out=wsums_sb[:], in_=wsums[:])
                    for k in range(KT):
                        w = wkp.tile([128, DIM], BF16, tag="wk", name=f"wk_{k}")
                        nc.sync.dma_start(out=w[:],
                                          in_=Wk[k * 128:(k + 1) * 128, :])
                        wk_t.append(w)
                    nc.sync.dma_start(out=cos_sb[:], in_=cosd[:])
                    nc.sync.dma_start(out=sin_sb[:], in_=sind[:])
                    for dram, sb in ((bq, bq_sb), (bk, bk_sb), (bp, bp_sb),
                                     (b1, b1_sb), (b2, b2_sb)):
                        nc.sync.dma_start(out=sb[:],
                                          in_=dram.rearrange("(a p) -> p a",
                                                             p=128))
                    for k in range(KT):
                        for hh in range(2):
                            xsq = xsqp.tile([128, 1024], BF16, tag="xsq")
                            hcs = slice(hh * 1024, (hh + 1) * 1024)
                            nc.scalar.square(out=xsq[:],
                                             in_=xbf_tiles[k][:, hcs])
                            for nn in range(2):
                                n = 2 * hh + nn
                                cs = slice(n * 512, (n + 1) * 512)
                                nc.tensor.matmul(ps_sums[n][:], ones_bf[:],
                                                 xbf_tiles[k][:, cs],
                                                 start=(k == 0),
                                                 stop=(k == KT - 1))
                                nc.tensor.matmul(ps_sqs[n][:], ones_bf[:],
                                                 xsq[:, nn * 512:(nn + 1) * 512],
                                                 start=(k == 0),
                                                 stop=(k == KT - 1))

                    rb_full = bcastp.tile([128, T], BF16)
                    rs_row = statp.tile([1, T], BF16, tag="rs_row")
                    # -mean needs only the sum stats: get it out first so the
                    # K/Q/V chain-tail augment matmuls unblock early
                    for n in range(NCH):
                        cs = slice(n * 512, (n + 1) * 512)
                        nc.scalar.mul(out=negm_row[0:1, cs], in_=ps_sums[n][:],
                                      mul=-1.0 / DIM)
                    for n in range(NCH):
                        cs = slice(n * 512, (n + 1) * 512)
                        rs_bf, nm_bf = _ln_finalize(nc, statp, ps_sums[n],
                                                    ps_sqs[n], eps_sb, "ln1")
                        nc.vector.tensor_copy(rs_row[0:1, cs], rs_bf[:])
                        psb = ps_stp.tile([128, 512], F32, tag="ps_sum",
                                          name="ps_bc_r")
                        nc.tensor.matmul(psb[:], ones_row_bf[:], rs_bf[:])
                        nc.scalar.copy(out=rb_full[:, cs], in_=psb[:])
                    # rstd per token on partitions (for the V scale): 16
                    # tiny K=1 transpose matmuls
                    for r in range(RT):
                        pst = ps_stp.tile([128, 1], F32, tag="ps_sq",
                                          name="ps_rscol")
                        nc.tensor.matmul(pst[:],
                                         rs_row[0:1, r * 128:(r + 1) * 128],
                                         ones_row_bf[0:1, 0:1])
                        nc.vector.tensor_copy(rs_col[:, r:r + 1], pst[:])
                    # fold rstd into the rope tables (in place)
                    nc.vector.tensor_mul(out=cos_sb[:], in0=cos_sb[:],
                                         in1=rb_full[:])
                    nc.vector.tensor_mul(out=sin_sb[:], in0=sin_sb[:],
                                         in1=rb_full[:])

                # ---- Phase 2a: K and Q (rope'd, feature-major) ----
                # Raw psum evacuation on the idle ScalarE; the matmul chains
                # depend only on raw x + weights, so the PE is busy while the
                # stats finalize; only the chain-tail augment matmul and the
                # rope multiplies wait for the fold tables.
                def rope_tile(ropep, dst, raw, cols):
                    sh = ropep.tile([128, 512], BF16, tag="rope_sh",
                                    name="rope_sh")
                    nc.vector.stream_shuffle(out=sh[:], in_=raw, mask=swap_mask)
                    nc.vector.tensor_mul(out=raw, in0=raw,
                                         in1=cos_sb[:, cols])
                    nc.vector.tensor_mul(out=sh[:], in0=sh[:],
                                         in1=sin_sb[:, cols])
                    nc.vector.tensor_add(out=dst, in0=raw, in1=sh[:])

                with (
                    tc.tile_pool(name="wq", bufs=KT) as wqp,
                    tc.tile_pool(name="rope", bufs=2) as ropep,
                    tc.tile_pool(name="ps_qk", bufs=3, space="PSUM") as ps_qkp,
                ):
                    for k in range(KT):
                        w = wqp.tile([128, DIM], BF16, tag="wq", name=f"wq_{k}")
                        nc.sync.dma_start(out=w[:],
                                          in_=Wq[k * 128:(k + 1) * 128, :])
                        wq_t.append(w)
                    for m in range(KT):
                        ms = slice(m * 128, (m + 1) * 128)
                        ksb = ksbp.tile([128, T], BF16, tag="ksb")
                        for n in range(NCH):
                            cs = slice(n * 512, (n + 1) * 512)
                            ps = ps_qkp.tile([128, 512], F32, tag="ps_k",
                                             name="ps_k")
                            for k in range(KT):
                                nc.tensor.matmul(ps[:], wk_t[k][:, ms],
                                                 xbf_tiles[k][:, cs],
                                                 start=(k == 0), stop=False)
                            nc.tensor.matmul(ps[:],
                                             wsums_sb[0:1, DIM + m * 128:
                                                      DIM + (m + 1) * 128],
                                             negm_row[0:1, cs],
                                             start=False, stop=True)
                            raw = ropep.tile([128, 512], BF16, tag="rope_raw",
                                             name="rope_raw")
                            nc.scalar.activation(out=raw[:], in_=ps[:],
                                                 func=AF.Identity,
                                                 bias=bk_sb[:, m:m + 1])
                            rope_tile(ropep, ksb[:, cs], raw[:], cs)
                        k_tiles.append(ksb)

                        qsb = qsbp.tile([128, QBLK], BF16, tag="qsb")
                        ps = ps_qkp.tile([128, 512], F32, tag="ps_q",
                                         name="ps_q")
                        for k in range(KT):
                            nc.tensor.matmul(ps[:], wq_t[k][:, ms],
                                             xbf_tiles[k][:, 0:QBLK],
                                             start=(k == 0), stop=False)
                        nc.tensor.matmul(ps[:],
                                         wsums_sb[0:1, m * 128:(m + 1) * 128],
                                         negm_row[0:1, 0:QBLK],
                                         start=False, stop=True)
                        raw = ropep.tile([128, 512], BF16, tag="rope_raw",
                                         name="rope_raw")
                        nc.scalar.activation(out=raw[:], in_=ps[:],
                                             func=AF.Identity,
                                             bias=bq_sb[:, m:m + 1])
                        rope_tile(ropep, qsb[:], raw[:], slice(0, QBLK))
                        q_tiles.append(qsb)

                wk_ctx.close()

                # ---- Phase 2b: V row-major with interleaved ones columns ----
                with (
                    tc.tile_pool(name="wv", bufs=KT) as wvp,
                    tc.tile_pool(name="ps_v", bufs=2, space="PSUM") as ps_vp,
                ):
                    wv_t = []
                    for k in range(KT):
                        w = wvp.tile([128, DIM], BF16, tag="wv", name=f"wv_{k}")
                        nc.sync.dma_start(out=w[:],
                                          in_=Wv[k * 128:(k + 1) * 128, :])
                        wv_t.append(w)
                    for r in range(RT):
                        rs_ = slice(r * 128, (r + 1) * 128)
                        ps = ps_vp.tile([128, DIM], F32, tag="ps_v", name="ps_v")
                        for vh in range(2):
                            vs = slice(vh * 512, (vh + 1) * 512)
                            for k in range(KT):
                                nc.tensor.matmul(ps[:, vs],
                                                 xbf_tiles[k][:, rs_],
                                                 wv_t[k][:, vs],
                                                 start=(k == 0), stop=False)
                            nc.tensor.matmul(
                                ps[:, vs], negm_row[0:1, rs_],
                                wsums_sb[0:1, 2 * DIM + vh * 512:
                                         2 * DIM + (vh + 1) * 512],
                                start=False, stop=True)
                        vsb = vsbp.tile([128, HEADS * (HD + 1)], BF16,
                                        tag="vsb")
                        v3 = vsb[:].rearrange("p (h c) -> p h c", c=HD + 1)
                        nc.scalar.activation(
                            out=v3[:, :, 0:HD],
                            in_=ps[:].rearrange("p (h c) -> p h c", c=HD),
                            func=AF.Identity, scale=rs_col[:, r:r + 1])
                        nc.vector.memset(v3[:, :, HD:HD + 1], 1.0)
                        v_tiles.append(vsb)

            # ---- Phase 3: attention ----
            if True:
                av_tiles = []
                # 32 (key-tile, head-half) units per f; scores for 3 units
                # land in one 3-bank psum tile -> one [128,1536] exp; the AV
                # matmuls trail by one group so the PE never waits on the
                # exp in flight.
                units = [(kt, half) for kt in range(RT) for half in range(2)]
                groups = [units[i:i + 3] for i in range(0, len(units), 3)]
                with (
                    tc.tile_pool(name="esb", bufs=3) as esbp,
                    tc.tile_pool(name="asm", bufs=2) as asmp,
                    tc.tile_pool(name="ps_s", bufs=2, space="PSUM") as ps_sp,
                    tc.tile_pool(name="ps_av", bufs=2, space="PSUM") as ps_avp,
                ):
                    for f in range(HEADS // 2):
                        avsb = avp.tile([128, QBLK], BF16, tag="avsb")
                        ps_av = [ps_avp.tile([HD + 1, QBLK], F32, tag="ps_av",
                                             name=f"ps_av_{f}_{i}")
                                 for i in range(2)]

                        def av_group(e, g):
                            for j, (kt, half) in enumerate(g):
                                h = 2 * f + half
                                nc.tensor.matmul(
                                    ps_av[half][:],
                                    v_tiles[kt][:, h * (HD + 1):(h + 1) * (HD + 1)],
                                    e[:, j * QBLK:(j + 1) * QBLK],
                                    start=(kt == 0), stop=(kt == RT - 1))

                        pending = None
                        for g in groups:
                            gw = len(g) * QBLK
                            ps_s = ps_sp.tile([128, 3 * QBLK], F32, tag="ps_s",
                                              name="ps_s")
                            for j, (kt, half) in enumerate(g):
                                kcs = slice(kt * 128, (kt + 1) * 128)
                                hs = slice(half * HD, (half + 1) * HD)
                                nc.tensor.matmul(
                                    ps_s[:, j * QBLK:(j + 1) * QBLK],
                                    k_tiles[f][hs, kcs], q_tiles[f][hs, :])
                            e = esbp.tile([128, 3 * QBLK], BF16, tag="esb",
                                          name="esb")
                            nc.scalar.activation(out=e[:, 0:gw],
                                                 in_=ps_s[:, 0:gw],
                                                 func=AF.Exp,
                                                 scale=float(1.0 / np.sqrt(HD)))
                            if pending is not None:
                                av_group(*pending)
                            pending = (e, g)
                        av_group(*pending)

                        for half in range(2):
                            den = asmp.tile([1, QBLK], F32, tag="den",
                                            name="den")
                            nc.vector.tensor_copy(den[:],
                                                  ps_av[half][HD:HD + 1, :])
                            r_row = asmp.tile([1, QBLK], F32, tag="r_row",
                                              name="r_row")
                            nc.vector.reciprocal_approx_fast(
                                out=r_row[:], in_=den[:])
                            r_bf = asmp.tile([1, QBLK], BF16, tag="r_bf",
                                             name="r_bf")
                            nc.vector.tensor_copy(r_bf[:], r_row[:])
                            av_un = asmp.tile([HD, QBLK], BF16, tag="av_un",
                                              name="av_un")
                            nc.vector.tensor_copy(av_un[:], ps_av[half][0:HD, :])
                            # broadcast 1/den back into the (now dead) av
                            # bank rows 0:HD -- no extra psum banks needed
                            nc.tensor.matmul(ps_av[half][0:HD, :],
                                             ones_row_bf[:, 0:HD], r_bf[:])
                            nc.vector.tensor_mul(
                                out=avsb[half * HD:(half + 1) * HD, :],
                                in0=av_un[:], in1=ps_av[half][0:HD, :])
                        av_tiles.append(avsb)

                kvq_ctx.close()

                # ---- Phase 4: proj + bias + residual (h1 in bf16) ----
                # w1 pool opens here so the 8MB W1 prefetch overlaps
                # proj+LN2; it stays open through FFN1/FFN2.
                w1_ctx = ExitStack()
                w1p = w1_ctx.enter_context(tc.tile_pool(name="w1", bufs=KT))
                ln2bc_ctx = ExitStack()
                bcast2p = ln2bc_ctx.enter_context(
                    tc.tile_pool(name="ln2bc", bufs=1))
                ln2_ctx = ExitStack()
                hsqp = ln2_ctx.enter_context(tc.tile_pool(name="hsq", bufs=1))
                stat2p = ln2_ctx.enter_context(tc.tile_pool(name="stats2",
                                                            bufs=1))
                ps_st2p = ln2_ctx.enter_context(
                    tc.tile_pool(name="ps_st2", bufs=2, space="PSUM"))
                ps2_sum = ps_st2p.tile([1, 512], F32, tag="ps_sum2",
                                       name="ps_sum2")
                ps2_sq = ps_st2p.tile([1, 512], F32, tag="ps_sq2",
                                      name="ps_sq2")
                with (
                    tc.tile_pool(name="wp", bufs=KT) as wpp,
                    tc.tile_pool(name="xq", bufs=2) as xqp,
                    tc.tile_pool(name="ps_p", bufs=3, space="PSUM") as ps_pp,
                ):
                    wp_t = []
                    for k in range(KT):
                        w = wpp.tile([128, DIM], BF16, tag="wp", name=f"wp_{k}")
                        nc.sync.dma_start(out=w[:],
                                          in_=Wp[k * 128:(k + 1) * 128, :])
                        wp_t.append(w)
                    w1_t = []
                    for k in range(KT):
                        w = w1p.tile([128, 4 * DIM], BF16, tag="w1",
                                     name=f"w1_{k}")
                        nc.sync.dma_start(out=w[:],
                                          in_=W1[k * 128:(k + 1) * 128, :])
                        w1_t.append(w)
                    for m in range(KT):
                        ms = slice(m * 128, (m + 1) * 128)
                        xq = xqp.tile([128, QBLK], F32, tag="xq", name="xq")
                        nc.gpsimd.dma_start(out=xq[:], in_=xT[ms, 0:QBLK])
                        ps = ps_pp.tile([128, QBLK], F32, tag="ps_p", name="ps_p")
                        for k in range(KT):
                            nc.tensor.matmul(ps[:], wp_t[k][:, ms], av_tiles[k][:],
                                             start=(k == 0), stop=(k == KT - 1))
                        h1 = h1p.tile([128, QBLK], BF16, tag="h1")
                        nc.vector.scalar_tensor_tensor(
                            out=h1[:], in0=ps[:], scalar=bp_sb[:, m:m + 1],
                            in1=xq[:], op0=OP.add, op1=OP.add)
                        h1_tiles.append(h1)
                        hsq = hsqp.tile([128, QBLK], BF16, tag="hsq")
                        nc.scalar.square(out=hsq[:], in_=h1[:])
                        nc.tensor.matmul(ps2_sum[:], ones_bf[:], h1[:],
                                         start=(m == 0), stop=(m == KT - 1))
                        nc.tensor.matmul(ps2_sq[:], ones_bf[:], hsq[:],
                                         start=(m == 0), stop=(m == KT - 1))

                # ---- Phase 5: LN2 finalize + apply ----
                rs_bf, nm_bf = _ln_finalize(nc, stat2p, ps2_sum,
                                            ps2_sq, eps_sb, "ln2")
                rb2 = bcast2p.tile([128, QBLK], BF16)
                nm2 = bcast2p.tile([128, QBLK], BF16)
                psb = ps_st2p.tile([128, 512], F32, tag="ps_sum2",
                                   name="ps_bc2r")
                nc.tensor.matmul(psb[:], ones_row_bf[:], rs_bf[:])
                nc.scalar.copy(out=rb2[:], in_=psb[:])
                psb2 = ps_st2p.tile([128, 512], F32, tag="ps_sq2",
                                    name="ps_bc2m")
                nc.tensor.matmul(psb2[:], ones_row_bf[:], nm_bf[:])
                nc.scalar.copy(out=nm2[:], in_=psb2[:])
                ln2_ctx.close()
                with tc.tile_pool(name="nx2", bufs=KT) as nx2p:
                    nx2_tiles = []
                    for k in range(KT):
                        nx2 = nx2p.tile([128, QBLK], BF16, tag="nx2")
                        nc.vector.tensor_add(out=nx2[:], in0=h1_tiles[k][:],
                                             in1=nm2[:])
                        nc.vector.tensor_mul(out=nx2[:], in0=nx2[:],
                                             in1=rb2[:])
                        nx2_tiles.append(nx2)

                    # ---- Phase 6: FFN1 + SiLU ----
                    with tc.tile_pool(name="hs", bufs=4 * KT) as hsp:
                        hs_tiles = []
                        with tc.tile_pool(name="ps_f", bufs=3,
                                          space="PSUM") as ps_fp:
                            for m in range(4 * KT):
                                ms = slice(m * 128, (m + 1) * 128)
                                ps = ps_fp.tile([128, QBLK], F32, tag="ps_f",
                                                name="ps_f")
                                for k in range(KT):
                                    nc.tensor.matmul(ps[:], w1_t[k][:, ms],
                                                     nx2_tiles[k][:],
                                                     start=(k == 0),
                                                     stop=(k == KT - 1))
                                hs = hsp.tile([128, QBLK], BF16, tag="hs",
                                              name="hs")
                                nc.scalar.activation(out=hs[:], in_=ps[:],
                                                     func=AF.Silu,
                                                     bias=b1_sb[:, m:m + 1])
                                hs_tiles.append(hs)

                        # ---- Phase 7: FFN2 + bias + residual ----
                        with (
                            tc.tile_pool(name="w2", bufs=6) as w2p,
                            tc.tile_pool(name="osb", bufs=2) as osbp,
                            tc.tile_pool(name="ps_o", bufs=KT,
                                         space="PSUM") as ps_op,
                        ):
                            ps_o = [ps_op.tile([128, QBLK], F32, tag="ps_o",
                                               name=f"ps_o_{i}")
                                    for i in range(KT)]
                            for k in range(4 * KT):
                                w2 = w2p.tile([128, DIM], BF16, tag="w2",
                                              name="w2")
                                nc.sync.dma_start(
                                    out=w2[:], in_=W2[k * 128:(k + 1) * 128, :])
                                for m in range(KT):
                                    nc.tensor.matmul(
                                        ps_o[m][:], w2[:, m * 128:(m + 1) * 128],
                                        hs_tiles[k][:],
                                        start=(k == 0), stop=(k == 4 * KT - 1))
                            for m in range(KT):
                                osb = osbp.tile([128, QBLK], F32, tag="osb",
                                                name="osb")
                                nc.vector.scalar_tensor_tensor(
                                    out=osb[:], in0=ps_o[m][:],
                                    scalar=b2_sb[:, m:m + 1],
                                    in1=h1_tiles[m][:], op0=OP.add, op1=OP.add)
                                nc.sync.dma_start(
                                    out=outT[m * 128:(m + 1) * 128, :],
                                    in_=osb[:])

                ln2bc_ctx.close()
                w1_ctx.close()

    nc.compile()
    return nc


_CACHE = {}


def _host_prep(inputs):
    g1 = np.asarray(inputs["ln1_g"], np.float32)
    b1v = np.asarray(inputs["ln1_b"], np.float32)
    g2 = np.asarray(inputs["ln2_g"], np.float32)
    b2v = np.asarray(inputs["ln2_b"], np.float32)
    W_qkv = np.asarray(inputs["W_qkv"], np.float32)
    b_qkv = np.asarray(inputs["b_qkv"], np.float32)
    W_proj = np.asarray(inputs["W_proj"], np.float32)
    b_proj = np.asarray(inputs["b_proj"], np.float32)
    W1 = np.asarray(inputs["W_ffn1"], np.float32)
    bf1 = np.asarray(inputs["b_ffn1"], np.float32)
    W2 = np.asarray(inputs["W_ffn2"], np.float32)
    bf2 = np.asarray(inputs["b_ffn2"], np.float32)

    Wf = g1[:, None] * W_qkv
    bf = b1v @ W_qkv + b_qkv
    Wq_, Wk_, Wv_ = Wf[:, :DIM], Wf[:, DIM:2 * DIM], Wf[:, 2 * DIM:]
    bq_, bk_, bv_ = bf[:DIM], bf[DIM:2 * DIM], bf[2 * DIM:]

    perm = np.empty(HD, np.int64)
    perm[0::2] = np.arange(HD // 2)
    perm[1::2] = np.arange(HD // 2) + HD // 2
    full_perm = np.concatenate([h * HD + perm for h in range(HEADS)])
    Wq_ = Wq_[:, full_perm]; bq_ = bq_[full_perm]
    Wk_ = Wk_[:, full_perm]; bk_ = bk_[full_perm]

    inv_freq = 1.0 / (ROPE_THETA ** (np.arange(0, HD, 2, dtype=np.float32) / HD))
    pos = np.arange(T, dtype=np.float32)
    ang = pos[None, :] * inv_freq[:, None]
    cosv = np.cos(ang).astype(np.float32)
    sinv = np.sin(ang).astype(np.float32)
    cos64 = np.repeat(cosv, 2, axis=0)
    sin64 = np.repeat(sinv, 2, axis=0).copy()
    sin64[0::2] *= -1.0
    cos2 = np.concatenate([cos64, cos64], axis=0).astype(_bf)
    sin2 = np.concatenate([sin64, sin64], axis=0).astype(_bf)

    bp_eff = b_proj + bv_ @ W_proj
    W1f = g2[:, None] * W1
    b1_eff = bf1 + b2v @ W1

    # column sums of the (gamma-folded, permuted) qkv weights for the
    # rank-1 -mean correction matmuls
    wsum_all = Wf.sum(axis=0)
    wsumq = wsum_all[:DIM][full_perm]
    wsumk = wsum_all[DIM:2 * DIM][full_perm]
    wsumv = wsum_all[2 * DIM:]
    wsums = np.concatenate([wsumq, wsumk, wsumv])[None, :].astype(_bf)

    c = np.ascontiguousarray
    return dict(
        Wq=c(Wq_.astype(_bf)), Wk=c(Wk_.astype(_bf)), Wv=c(Wv_.astype(_bf)),
        Wp=c(W_proj.astype(_bf)), W1=c(W1f.astype(_bf)), W2=c(W2.astype(_bf)),
        bq=c(bq_), bk=c(bk_), bp=c(bp_eff), b1=c(b1_eff), b2=c(bf2),
        wsums=c(wsums), cos2=cos2, sin2=sin2,
    )


def make_in_maps(inputs):
    P = _host_prep(inputs)
    x = np.asarray(inputs["x"], np.float32)
    shared = {k: P[k] for k in ("Wq", "Wk", "Wv", "Wp", "W1", "W2",
                                "bq", "bk", "bp", "b1", "b2", "wsums")}
    in_maps = []
    for c in range(NCORES):
        b = c // CPB
        qb = c % CPB
        roll = -qb * QBLK
        xTr = np.ascontiguousarray(np.roll(x[b].T, roll, axis=1))
        cosd = np.ascontiguousarray(np.roll(P["cos2"], roll, axis=1))
        sind = np.ascontiguousarray(np.roll(P["sin2"], roll, axis=1))
        in_maps.append(dict(shared, xT=xTr, cosd=cosd, sind=sind))
    return in_maps


def assemble_out(results):
    out = np.empty((B, T, DIM), np.float32)
    for c in range(NCORES):
        b = c // CPB
        qb = c % CPB
        out[b, qb * QBLK:(qb + 1) * QBLK, :] = results[c]["outT"].T
    return out


def get_program():
    if "nc" not in _CACHE:
        _CACHE["nc"] = _build_program()
    return _CACHE["nc"]


def kernel(**inputs):
    nc = get_program()
    in_maps = make_in_maps(inputs)
    res = run_bass_kernel_spmd(nc, in_maps, list(range(NCORES)))
    return assemble_out(res.results)
